# revision 45
# baseline (speedup 1.0000x reference)
"""
Trainium2 Bass kernel for nn_EquivariantProductBasisBlock.

Math (per node n, channel c):
    s   = feats[n,c,0];  v = feats[n,c,1:4]
    v2  = |v|^2 ;  s2 = s^2
    w0p[n,c] = w_paths0[spec(n),p,c]   (attrs are one-hot -> table lookup)
    w1p[n,c] = w_paths1[spec(n),p,c]
    out0 = s*w00 + s2*w01 + s3*w03 + v2*(w02 + s*w04)
    c1   = w10 + s*w11 + s2*w12 + v2*w13
    y0   = out0 @ W_lin0 ;  y1_i = (c1 * v_i) @ W_lin1
    out  = [y0 | interleave_i(y1_i)]

Key design (see kernel_baseline.py docstring for the original rationale):
  * Host sorts nodes by species, pads to SUPER=1280 multiples -> every
    super-tile is species-uniform; path weights become per-partition
    scalars [128,1] for DVE tensor_scalar ops.
  * All elementwise tiles fp16 (~1.2e-3 scale-rel err; gate is 2e-2
    scale-relative absmax).  fp8 anywhere fails: lhsT/rhs fp8 matmul
    quantization alone lands at ~2e-2 == the gate.
  * v2 precomputed on host as a 5th input plane (on-chip it needs 3
    squares + 2 adds and no engine has room; tensor_reduce can't
    pre-square).
  * 0e path: w00 folded into one per-super scaled W0 (on-device [C,C]
    DVE op); s2@diag(w01)W0 and s3@diag(w03)W0 matmuls kept on PE --
    folding them into DVE (v8) raised DVE busy 77->97us and DOUBLED
    power throttle: total 140-150us.  PE is the right engine for them.
  * Elementwise chain: ONLY tensor_scalar (4x DVE mode ~0.6us/1280)
    and tensor_tensor (2x ~0.82us); scalar_tensor_tensor is 1x mode
    (1543ns measured) -- any stt "fusion" is a net LOSS.
  * 1o chain in Horner form avoids depending on Act's Square until the
    late s3 op, decoupling DVE from Act's eviction FIFO.
  * DMA queue split: feats on the sync HWDGE ring (Q1), output on the
    scalar HWDGE ring (Q10); joint active BW ~350GB/s (per-NC HBM limit
    358).  Pure-DMA probe of this traffic: 101us incl ~12us barriers.
  * POWER THROTTLE dominates run-to-run variance: chip duty-cycles to
    k=4/n=8 in 3.4us quanta once hot (30-90us of throttle observed).
    Bursty DMA triggers it: 3.3MB paired loads +20us vs HALF-SUPER
    0.82MB loads (best).  Quarter loads and split per-super flushes
    also regressed (+15us) -- one whole-super 1.31MB flush is optimal.
  * Output DRAM layout [128, nsuper*NSUB, 4C] matches SBUF partition
    order -> fully contiguous per-partition writes (host un-permutes
    for free).
  * Output fp16; Act evicts PSUM->SBUF fp16 (1114ns/pair; DVE does the
    same [128,2,512] evict in 1224ns -- roughly equal, NOT faster);
    last TWO supers split evictions across Act+DVE (drain region, DVE
    otherwise idle; DVE takes the 3 even groups) and the last super's
    flush triggers ride the idle Sync engine/Q1.  Extending the split
    to three supers regressed.  Hoisting the per-super w0 scalings to
    the fill phase also regressed (+18us: 30 ops at DVE FIFO head
    stall super 0 behind the const DMAs).
  * Uniform 10x1280 tiles: half-size tiles at the ends (v5) regressed
    ~+15us.  Engine busy/super: DVE 7.7us (3ts+7tt+3 small), Act ~7
    (Square + 5 evicts + triggers), PE ~7.5 (70 matmuls), DMA-in 6.7 +
    DMA-out 5.4 on parallel queues; cadence ~9-10us.
  * Best measured: 112.7-113.4us (hot chip up to ~124; pure-DMA floor
    for the 29.6MB at play is ~101us, incl ~12us framework barriers).
    io bufs=5 / ewz=4 / ob=4; io=6 overflows SBUF.
"""

import sys

sys.path.insert(0, "/opt/trn_rl_repo")

from contextlib import ExitStack

import numpy as np

import concourse.bass as bass
import concourse.tile as tile
from concourse import bacc, mybir
from concourse.bass_utils import run_bass_kernel_spmd

N_CORES = 8
N_NODES = 100000
C = 128
S = 10
SUB = 128
NSUB = 10
SUPER = SUB * NSUB           # 1280 nodes per species-uniform tile
F32 = mybir.dt.float32
F16 = mybir.dt.float16


def build_bass(nsuper):
    per_core = nsuper * SUPER
    nc = bacc.Bacc()
    featsT = nc.dram_tensor("featsT", (5, C, per_core), F16, kind="ExternalInput")
    wsel = nc.dram_tensor("wsel", (C, nsuper * 9), F32, kind="ExternalInput")
    wl0 = nc.dram_tensor("wl0", (C, C), F16, kind="ExternalInput")
    wl1 = nc.dram_tensor("wl1", (C, C), F16, kind="ExternalInput")
    out = nc.dram_tensor("out", (SUB, nsuper * NSUB, 4 * C), F16, kind="ExternalOutput")

    with tile.TileContext(nc) as tc, ExitStack() as ctx:
        _body(ctx, tc, featsT, wsel, wl0, wl1, out, nsuper)
    nc.finalize()
    return nc


def _body(ctx, tc, featsT, wsel, wl0, wl1, out, nsuper):
    nc = tc.nc
    mult = mybir.AluOpType.mult
    add = mybir.AluOpType.add
    SQ = mybir.ActivationFunctionType.Square

    const = ctx.enter_context(tc.tile_pool(name="const", bufs=1))
    io = ctx.enter_context(tc.tile_pool(name="io", bufs=5))
    ob = ctx.enter_context(tc.tile_pool(name="ob", bufs=4))
    ew = ctx.enter_context(tc.tile_pool(name="ew", bufs=3))
    ewz = ctx.enter_context(tc.tile_pool(name="ewz", bufs=4))
    wp = ctx.enter_context(tc.tile_pool(name="wp", bufs=2))
    ps = ctx.enter_context(tc.tile_pool(name="ps", bufs=3, space="PSUM"))

    # const loads on the scalar queue (HWDGE, idle at t=0) so the first
    # feats DMA (sync queue) starts immediately
    wsel_sb = const.tile([C, nsuper * 9], F32)
    nc.scalar.dma_start(out=wsel_sb, in_=wsel[:, :])
    wl0_sb = const.tile([C, C], F16)
    nc.scalar.dma_start(out=wl0_sb, in_=wl0[:, :])
    wl1_sb = const.tile([C, C], F16)
    nc.scalar.dma_start(out=wl1_sb, in_=wl1[:, :])

    # load + square emitted ONE SUPER AHEAD: the Act Square for ci+1
    # then sits BEFORE evicts(ci) in Act's FIFO, so s2 is always ready
    # when DVE reaches s3 (the one cross-engine coupling in the chain).
    # Half-super (0.82MB) transfers on the sync ring (Q1): smoother DMA
    # power draw -- bursty 3.3MB paired loads measured +20us throttle.
    def emit_load_sq(cj):
        n0 = cj * SUPER
        fT = io.tile([C, 5, SUPER], F16, tag="fT", name="fT")
        H = SUPER // 2
        nc.sync.dma_start(
            out=fT[:, :, :H],
            in_=featsT[:, :, n0 : n0 + H].rearrange("k c n -> c k n"),
        )
        nc.sync.dma_start(
            out=fT[:, :, H:],
            in_=featsT[:, :, n0 + H : n0 + SUPER].rearrange("k c n -> c k n"),
        )
        s2n = ewz.tile([C, SUPER], F16, tag="s2", name="s2")
        nc.scalar.activation(out=s2n, in_=fT[:, 0, :], func=SQ)
        return fT, s2n

    fT_cur, s2_cur = emit_load_sq(0)
    for ci in range(nsuper):
        fT, s2 = fT_cur, s2_cur
        if ci + 1 < nsuper:
            fT_cur, s2_cur = emit_load_sq(ci + 1)
        s = fT[:, 0, :]
        vx = fT[:, 1, :]
        vy = fT[:, 2, :]
        vz = fT[:, 3, :]
        v2 = fT[:, 4, :]

        def wcol(j):
            return wsel_sb[:, ci * 9 + j : ci * 9 + j + 1]

        def t16(tag, pool=ew):
            return pool.tile([C, SUPER], F16, tag=tag, name=tag)

        # ---- per-super scaled W0 matrices (replaces 98KB/super DMA):
        #      w0a = diag(w00) @ W0  etc., per-partition row scaling
        w0a = wp.tile([C, C], F16, tag="w0a", name="w0a")
        nc.vector.tensor_scalar(out=w0a, in0=wl0_sb, scalar1=wcol(0), scalar2=None,
                                op0=mult)
        w0b = wp.tile([C, C], F16, tag="w0b", name="w0b")
        nc.vector.tensor_scalar(out=w0b, in0=wl0_sb, scalar1=wcol(1), scalar2=None,
                                op0=mult)
        w0d = wp.tile([C, C], F16, tag="w0d", name="w0d")
        nc.vector.tensor_scalar(out=w0d, in0=wl0_sb, scalar1=wcol(3), scalar2=None,
                                op0=mult)

        # ---- DVE (ts=4x mode ~0.6us, tt=2x ~0.82us; stt is 1x -- avoid).
        # Ordered so PE operands appear ASAP (b/ZC first), and the 1o
        # chain is Horner-form so it does NOT wait on Act's square:
        # the only s2 consumers (s3, matmul) sit late in the chain.
        b = t16("b")    # s*w04 + w02
        nc.vector.tensor_scalar(out=b, in0=s, scalar1=wcol(4), scalar2=wcol(2),
                                op0=mult, op1=add)
        ZC = t16("ZC", ewz)
        nc.vector.tensor_tensor(out=ZC, in0=v2, in1=b, op=mult)
        # 0e cubic (needs s2 from Act; 3rd op so PE's accum chain unblocks)
        s3 = t16("s3", ewz)
        nc.vector.tensor_tensor(out=s3, in0=s2, in1=s, op=mult)
        # 1o: c1 = (w12 s + w11) s + (w13 v2 + w10)
        q1 = t16("q1")
        nc.vector.tensor_scalar(out=q1, in0=s, scalar1=wcol(7), scalar2=wcol(6),
                                op0=mult, op1=add)
        q2 = t16("q2")
        nc.vector.tensor_tensor(out=q2, in0=q1, in1=s, op=mult)
        wv = t16("wv")  # v2*w13 + w10
        nc.vector.tensor_scalar(out=wv, in0=v2, scalar1=wcol(8), scalar2=wcol(5),
                                op0=mult, op1=add)
        c1 = t16("c1")
        nc.vector.tensor_tensor(out=c1, in0=q2, in1=wv, op=add)
        Z1x = t16("Z1x", ewz)
        nc.vector.tensor_tensor(out=Z1x, in0=c1, in1=vx, op=mult)
        Z1y = t16("Z1y", ewz)
        nc.vector.tensor_tensor(out=Z1y, in0=c1, in1=vy, op=mult)
        Z1z = t16("Z1z", ewz)
        nc.vector.tensor_tensor(out=Z1z, in0=c1, in1=vz, op=mult)

        # ---- final per-irrep linears + eviction ----
        out_sb = ob.tile([SUB, NSUB, 4 * C], F16, tag="out_sb")
        for h in range(NSUB // 2):
            y4 = ps.tile([SUB, 2, 4 * C], F32, tag="y")
            for q in range(2):
                t = 2 * h + q
                sl = bass.ts(t, SUB)
                nc.tensor.matmul(y4[:, q, 0:C], lhsT=s[:, sl], rhs=w0a,
                                 start=True, stop=False)
                nc.tensor.matmul(y4[:, q, 0:C], lhsT=ZC[:, sl], rhs=wl0_sb,
                                 start=False, stop=False)
                nc.tensor.matmul(y4[:, q, 0:C], lhsT=s2[:, sl], rhs=w0b,
                                 start=False, stop=False)
                nc.tensor.matmul(y4[:, q, 0:C], lhsT=s3[:, sl], rhs=w0d,
                                 start=False, stop=True)
                nc.tensor.matmul(y4[:, q, C:2 * C], lhsT=Z1x[:, sl], rhs=wl1_sb,
                                 start=True, stop=True)
                nc.tensor.matmul(y4[:, q, 2 * C:3 * C], lhsT=Z1y[:, sl], rhs=wl1_sb,
                                 start=True, stop=True)
                nc.tensor.matmul(y4[:, q, 3 * C:4 * C], lhsT=Z1z[:, sl], rhs=wl1_sb,
                                 start=True, stop=True)
            dst = out_sb[:, 2 * h : 2 * h + 2, :]
            if ci >= nsuper - 2 and h % 2 == 0:
                # last super: split the eviction chain across Act and the
                # (by then idle) DVE so the pipeline drain runs in parallel
                nc.vector.tensor_scalar(out=dst, in0=y4, scalar1=1.0,
                                        scalar2=None, op0=mult)
            else:
                nc.scalar.copy(out=dst, in_=y4)
            # last super: flush in thirds to shorten the pipeline drain.
            # Triggers ride the SYNC engine (idle in the drain; Act's
            # FIFO is full of evictions, and Q1 has no input left).
            if ci == nsuper - 1 and h in (1, 3, 4):
                lo = 0 if h == 1 else (4 if h == 3 else 8)
                nc.sync.dma_start(
                    out=out[:, ci * NSUB + lo : ci * NSUB + 2 * h + 2, :],
                    in_=out_sb[:, lo : 2 * h + 2, :],
                )
        # one contiguous 1.31MB flush per super on the scalar ring (Q10)
        if ci != nsuper - 1:
            nc.scalar.dma_start(
                out=out[:, ci * NSUB : (ci + 1) * NSUB, :],
                in_=out_sb[:, :, :],
            )


_NC_CACHE = {}


def _get_nc(nsuper):
    if nsuper not in _NC_CACHE:
        _NC_CACHE[nsuper] = build_bass(nsuper)
    return _NC_CACHE[nsuper]


def kernel(node_feats, node_attrs, w_paths0, w_paths1, W_lin0, W_lin1):
    n = node_feats.shape[0]
    assert n == N_NODES, n

    species = np.argmax(np.asarray(node_attrs), axis=1).astype(np.int64)
    counts = np.bincount(species, minlength=S)
    sup_sp = -(-counts // SUPER)                       # supers per species
    total_sup = int(sup_sp.sum())
    T = -(-total_sup // N_CORES) * N_CORES             # pad to multiple of 8
    nsuper = T // N_CORES
    padded_n = T * SUPER
    per_core = nsuper * SUPER

    # destination slot (species-sorted, super-padded) for each node
    off = np.zeros(S, np.int64)
    off[1:] = np.cumsum(sup_sp * SUPER)[:-1]
    order = np.argsort(species, kind="stable")
    dst = np.empty(n, np.int64)
    pos = 0
    for sp in range(S):
        n_s = int(counts[sp])
        dst[order[pos : pos + n_s]] = off[sp] + np.arange(n_s)
        pos += n_s

    # species of each super tile (padding supers read species 0 weights)
    sup_species = np.zeros(T, np.int64)
    sup_species[:total_sup] = np.repeat(np.arange(S), sup_sp)

    # input planes [5, C, padded_n] fp16: s, vx, vy, vz, |v|^2
    f = np.asarray(node_feats, np.float32)
    planes = np.zeros((5, C, padded_n), np.float16)
    planes[0][:, dst] = f[:, :, 0].T
    planes[1][:, dst] = f[:, :, 1].T
    planes[2][:, dst] = f[:, :, 2].T
    planes[3][:, dst] = f[:, :, 3].T
    v2 = f[:, :, 1] ** 2 + f[:, :, 2] ** 2 + f[:, :, 3] ** 2
    planes[4][:, dst] = v2.T

    # per-super path-weight scalars [C, T*9] fp32
    w0 = np.asarray(w_paths0, np.float32)              # [S, 5, C]
    w1 = np.asarray(w_paths1, np.float32)              # [S, 4, C]
    wtab = np.concatenate([w0, w1], axis=1)            # [S, 9, C]
    wsel = wtab[sup_species].transpose(2, 0, 1).reshape(C, T * 9)

    wl0 = np.asarray(W_lin0, np.float16)
    wl1 = np.asarray(W_lin1, np.float16)

    nc = _get_nc(nsuper)
    in_maps = []
    for k in range(N_CORES):
        c0 = k * per_core
        in_maps.append(
            {
                "featsT": np.ascontiguousarray(planes[:, :, c0 : c0 + per_core]),
                "wsel": np.ascontiguousarray(
                    wsel[:, k * nsuper * 9 : (k + 1) * nsuper * 9]
                ),
                "wl0": wl0,
                "wl1": wl1,
            }
        )
    res = run_bass_kernel_spmd(nc, in_maps, core_ids=list(range(N_CORES)))
    # device layout [128, nsuper*NSUB, 4C]: node (ci,t,p) -> [p, ci*NSUB+t, :]
    outs = [
        res.results[k]["out"].transpose(1, 0, 2).reshape(per_core, 4 * C)
        for k in range(N_CORES)
    ]
    full = np.concatenate(outs, axis=0)[dst].astype(np.float32)  # [n, 512]

    y0 = full[:, :C]
    y1 = full[:, C:].reshape(n, 3, C).transpose(0, 2, 1).reshape(n, 3 * C)
    return np.ascontiguousarray(np.concatenate([y0, y1], axis=1))


# revision 46
# speedup vs baseline: 1.0455x; 1.0455x over previous
"""
Trainium2 Bass kernel for nn_EquivariantProductBasisBlock.

Math (per node n, channel c):
    s   = feats[n,c,0];  v = feats[n,c,1:4]
    v2  = |v|^2 ;  s2 = s^2
    w0p[n,c] = w_paths0[spec(n),p,c]   (attrs are one-hot -> table lookup)
    w1p[n,c] = w_paths1[spec(n),p,c]
    out0 = s*w00 + s2*w01 + s3*w03 + v2*(w02 + s*w04)
    c1   = w10 + s*w11 + s2*w12 + v2*w13
    y0   = out0 @ W_lin0 ;  y1_i = (c1 * v_i) @ W_lin1
    out  = [y0 | interleave_i(y1_i)]

Key design (see kernel_baseline.py docstring for the original rationale):
  * Host sorts nodes by species, pads to SUPER=1280 multiples -> every
    super-tile is species-uniform; path weights become per-partition
    scalars [128,1] for DVE tensor_scalar ops.
  * All elementwise tiles fp16 (~1.2e-3 scale-rel err; gate is 2e-2
    scale-relative absmax).  fp8 anywhere fails: lhsT/rhs fp8 matmul
    quantization alone lands at ~2e-2 == the gate.
  * v2 precomputed on host as a 5th input plane (on-chip it needs 3
    squares + 2 adds and no engine has room; tensor_reduce can't
    pre-square).
  * 0e path: w00 folded into one per-super scaled W0 (on-device [C,C]
    DVE op); s2@diag(w01)W0 and s3@diag(w03)W0 matmuls kept on PE --
    folding them into DVE (v8) raised DVE busy 77->97us and DOUBLED
    power throttle: total 140-150us.  PE is the right engine for them.
  * Elementwise chain: ONLY tensor_scalar (4x DVE mode ~0.6us/1280)
    and tensor_tensor (2x ~0.82us); scalar_tensor_tensor is 1x mode
    (1543ns measured) -- any stt "fusion" is a net LOSS.
  * 1o chain in Horner form avoids depending on Act's Square until the
    late s3 op, decoupling DVE from Act's eviction FIFO.
  * DMA queue split: feats on the sync HWDGE ring (Q1), output on the
    scalar HWDGE ring (Q10); joint active BW ~350GB/s (per-NC HBM limit
    358).  Pure-DMA probe of this traffic: 101us incl ~12us barriers.
  * POWER THROTTLE dominates run-to-run variance: chip duty-cycles to
    k=4/n=8 in 3.4us quanta once hot (30-90us of throttle observed).
    Bursty DMA triggers it: 3.3MB paired loads +20us vs HALF-SUPER
    0.82MB loads (best).  Quarter loads and split per-super flushes
    also regressed (+15us) -- one whole-super 1.31MB flush is optimal.
  * Output DRAM layout [128, nsuper*NSUB, 4C] matches SBUF partition
    order -> fully contiguous per-partition writes (host un-permutes
    for free).
  * Output fp16; Act evicts PSUM->SBUF fp16 (1114ns/pair; DVE does the
    same [128,2,512] evict in 1224ns -- roughly equal, NOT faster);
    last TWO supers split evictions across Act+DVE (drain region, DVE
    otherwise idle; DVE takes the 3 even groups) and the last super's
    flush triggers ride the idle Sync engine/Q1.  Extending the split
    to three supers regressed.  Hoisting the per-super w0 scalings to
    the fill phase also regressed (+18us: 30 ops at DVE FIFO head
    stall super 0 behind the const DMAs).
  * Uniform 10x1280 tiles: half-size tiles at the ends (v5) regressed
    ~+15us.  Engine busy/super: DVE 7.7us (3ts+7tt+3 small), Act ~7
    (Square + 5 evicts + triggers), PE ~7.5 (70 matmuls), DMA-in 6.7 +
    DMA-out 5.4 on parallel queues; cadence ~9-10us.
  * Best measured: 112.7-113.4us (hot chip up to ~124; pure-DMA floor
    for the 29.6MB at play is ~101us, incl ~12us framework barriers).
    io bufs=5 / ewz=4 / ob=4; io=6 overflows SBUF.
"""

import sys

sys.path.insert(0, "/opt/trn_rl_repo")

from contextlib import ExitStack

import numpy as np

import concourse.bass as bass
import concourse.tile as tile
from concourse import bacc, mybir
from concourse.bass_utils import run_bass_kernel_spmd

N_CORES = 8
N_NODES = 100000
C = 128
S = 10
SUB = 128
NSUB = 10
SUPER = SUB * NSUB           # 1280 nodes per species-uniform tile
F32 = mybir.dt.float32
F16 = mybir.dt.float16


def build_bass(nsuper):
    per_core = nsuper * SUPER
    nc = bacc.Bacc()
    featsT = nc.dram_tensor("featsT", (5, C, per_core), F16, kind="ExternalInput")
    wsel = nc.dram_tensor("wsel", (C, nsuper * 9), F32, kind="ExternalInput")
    wl0 = nc.dram_tensor("wl0", (C, C), F16, kind="ExternalInput")
    wl1 = nc.dram_tensor("wl1", (C, C), F16, kind="ExternalInput")
    out = nc.dram_tensor("out", (SUB, nsuper * NSUB, 4 * C), F16, kind="ExternalOutput")

    with tile.TileContext(nc) as tc, ExitStack() as ctx:
        _body(ctx, tc, featsT, wsel, wl0, wl1, out, nsuper)
    nc.finalize()
    return nc


def _body(ctx, tc, featsT, wsel, wl0, wl1, out, nsuper):
    nc = tc.nc
    mult = mybir.AluOpType.mult
    add = mybir.AluOpType.add
    SQ = mybir.ActivationFunctionType.Square

    const = ctx.enter_context(tc.tile_pool(name="const", bufs=1))
    io = ctx.enter_context(tc.tile_pool(name="io", bufs=5))
    ob = ctx.enter_context(tc.tile_pool(name="ob", bufs=4))
    ew = ctx.enter_context(tc.tile_pool(name="ew", bufs=3))
    ewz = ctx.enter_context(tc.tile_pool(name="ewz", bufs=4))
    wp = ctx.enter_context(tc.tile_pool(name="wp", bufs=2))
    ps = ctx.enter_context(tc.tile_pool(name="ps", bufs=3, space="PSUM"))

    # const loads on the scalar queue (HWDGE, idle at t=0) so the first
    # feats DMA (sync queue) starts immediately
    wsel_sb = const.tile([C, nsuper * 9], F32)
    nc.scalar.dma_start(out=wsel_sb, in_=wsel[:, :])
    wl0_sb = const.tile([C, C], F16)
    nc.scalar.dma_start(out=wl0_sb, in_=wl0[:, :])
    wl1_sb = const.tile([C, C], F16)
    nc.scalar.dma_start(out=wl1_sb, in_=wl1[:, :])

    # per-super loads: smoother prefetch, 4-deep pipeline
    for ci in range(nsuper):
        n0 = ci * SUPER

        fT = io.tile([C, 5, SUPER], F16, tag="fT", name="fT")
        # feats rides the sync ring (Q1); output rides the scalar
        # ring (Q10) so in/out drain on independent HWDGE queues.
        # Half-super (0.82MB) transfers: smoother DMA power draw --
        # bursty 3.3MB paired loads measured +20us of extra throttle.
        H = SUPER // 2
        nc.sync.dma_start(
            out=fT[:, :, :H],
            in_=featsT[:, :, n0 : n0 + H].rearrange("k c n -> c k n"),
        )
        nc.sync.dma_start(
            out=fT[:, :, H:],
            in_=featsT[:, :, n0 + H : n0 + SUPER].rearrange("k c n -> c k n"),
        )
        s = fT[:, 0, :]
        vx = fT[:, 1, :]
        vy = fT[:, 2, :]
        vz = fT[:, 3, :]
        v2 = fT[:, 4, :]

        def wcol(j):
            return wsel_sb[:, ci * 9 + j : ci * 9 + j + 1]

        def t16(tag, pool=ew):
            return pool.tile([C, SUPER], F16, tag=tag, name=tag)

        # ---- per-super scaled W0 matrices (replaces 98KB/super DMA):
        #      w0a = diag(w00) @ W0  etc., per-partition row scaling
        w0a = wp.tile([C, C], F16, tag="w0a", name="w0a")
        nc.vector.tensor_scalar(out=w0a, in0=wl0_sb, scalar1=wcol(0), scalar2=None,
                                op0=mult)
        w0b = wp.tile([C, C], F16, tag="w0b", name="w0b")
        nc.vector.tensor_scalar(out=w0b, in0=wl0_sb, scalar1=wcol(1), scalar2=None,
                                op0=mult)
        w0d = wp.tile([C, C], F16, tag="w0d", name="w0d")
        nc.vector.tensor_scalar(out=w0d, in0=wl0_sb, scalar1=wcol(3), scalar2=None,
                                op0=mult)

        # ---- Activation engine: square (own SBUF port; GPSIMD shares
        #      ports with DVE, so Pool must stay off tensor ops entirely)
        s2 = t16("s2", ewz)
        nc.scalar.activation(out=s2, in_=s, func=SQ)

        # ---- DVE (ts=4x mode ~0.6us, tt=2x ~0.82us; stt is 1x -- avoid).
        # Ordered so PE operands appear ASAP (b/ZC first), and the 1o
        # chain is Horner-form so it does NOT wait on Act's square:
        # the only s2 consumers (s3, matmul) sit late in the chain.
        b = t16("b")    # s*w04 + w02
        nc.vector.tensor_scalar(out=b, in0=s, scalar1=wcol(4), scalar2=wcol(2),
                                op0=mult, op1=add)
        ZC = t16("ZC", ewz)
        nc.vector.tensor_tensor(out=ZC, in0=v2, in1=b, op=mult)
        # 0e cubic (needs s2 from Act; 3rd op so PE's accum chain unblocks)
        s3 = t16("s3", ewz)
        nc.vector.tensor_tensor(out=s3, in0=s2, in1=s, op=mult)
        # 1o: c1 = (w12 s + w11) s + (w13 v2 + w10)
        q1 = t16("q1")
        nc.vector.tensor_scalar(out=q1, in0=s, scalar1=wcol(7), scalar2=wcol(6),
                                op0=mult, op1=add)
        q2 = t16("q2")
        nc.vector.tensor_tensor(out=q2, in0=q1, in1=s, op=mult)
        wv = t16("wv")  # v2*w13 + w10
        nc.vector.tensor_scalar(out=wv, in0=v2, scalar1=wcol(8), scalar2=wcol(5),
                                op0=mult, op1=add)
        c1 = t16("c1")
        nc.vector.tensor_tensor(out=c1, in0=q2, in1=wv, op=add)
        Z1x = t16("Z1x", ewz)
        nc.vector.tensor_tensor(out=Z1x, in0=c1, in1=vx, op=mult)
        Z1y = t16("Z1y", ewz)
        nc.vector.tensor_tensor(out=Z1y, in0=c1, in1=vy, op=mult)
        Z1z = t16("Z1z", ewz)
        nc.vector.tensor_tensor(out=Z1z, in0=c1, in1=vz, op=mult)

        # ---- final per-irrep linears + eviction ----
        out_sb = ob.tile([SUB, NSUB, 4 * C], F16, tag="out_sb")
        for h in range(NSUB // 2):
            y4 = ps.tile([SUB, 2, 4 * C], F32, tag="y")
            for q in range(2):
                t = 2 * h + q
                sl = bass.ts(t, SUB)
                nc.tensor.matmul(y4[:, q, 0:C], lhsT=s[:, sl], rhs=w0a,
                                 start=True, stop=False)
                nc.tensor.matmul(y4[:, q, 0:C], lhsT=ZC[:, sl], rhs=wl0_sb,
                                 start=False, stop=False)
                nc.tensor.matmul(y4[:, q, 0:C], lhsT=s2[:, sl], rhs=w0b,
                                 start=False, stop=False)
                nc.tensor.matmul(y4[:, q, 0:C], lhsT=s3[:, sl], rhs=w0d,
                                 start=False, stop=True)
                nc.tensor.matmul(y4[:, q, C:2 * C], lhsT=Z1x[:, sl], rhs=wl1_sb,
                                 start=True, stop=True)
                nc.tensor.matmul(y4[:, q, 2 * C:3 * C], lhsT=Z1y[:, sl], rhs=wl1_sb,
                                 start=True, stop=True)
                nc.tensor.matmul(y4[:, q, 3 * C:4 * C], lhsT=Z1z[:, sl], rhs=wl1_sb,
                                 start=True, stop=True)
            dst = out_sb[:, 2 * h : 2 * h + 2, :]
            if ci >= nsuper - 2 and h % 2 == 0:
                # last super: split the eviction chain across Act and the
                # (by then idle) DVE so the pipeline drain runs in parallel
                nc.vector.tensor_scalar(out=dst, in0=y4, scalar1=1.0,
                                        scalar2=None, op0=mult)
            else:
                nc.scalar.copy(out=dst, in_=y4)
            # last super: flush in thirds to shorten the pipeline drain.
            # Triggers ride the SYNC engine (idle in the drain; Act's
            # FIFO is full of evictions, and Q1 has no input left).
            if ci == nsuper - 1 and h in (1, 3, 4):
                lo = 0 if h == 1 else (4 if h == 3 else 8)
                nc.sync.dma_start(
                    out=out[:, ci * NSUB + lo : ci * NSUB + 2 * h + 2, :],
                    in_=out_sb[:, lo : 2 * h + 2, :],
                )
        # one contiguous 1.31MB flush per super on the scalar ring (Q10)
        if ci != nsuper - 1:
            nc.scalar.dma_start(
                out=out[:, ci * NSUB : (ci + 1) * NSUB, :],
                in_=out_sb[:, :, :],
            )


_NC_CACHE = {}


def _get_nc(nsuper):
    if nsuper not in _NC_CACHE:
        _NC_CACHE[nsuper] = build_bass(nsuper)
    return _NC_CACHE[nsuper]


def kernel(node_feats, node_attrs, w_paths0, w_paths1, W_lin0, W_lin1):
    n = node_feats.shape[0]
    assert n == N_NODES, n

    species = np.argmax(np.asarray(node_attrs), axis=1).astype(np.int64)
    counts = np.bincount(species, minlength=S)
    sup_sp = -(-counts // SUPER)                       # supers per species
    total_sup = int(sup_sp.sum())
    T = -(-total_sup // N_CORES) * N_CORES             # pad to multiple of 8
    nsuper = T // N_CORES
    padded_n = T * SUPER
    per_core = nsuper * SUPER

    # destination slot (species-sorted, super-padded) for each node
    off = np.zeros(S, np.int64)
    off[1:] = np.cumsum(sup_sp * SUPER)[:-1]
    order = np.argsort(species, kind="stable")
    dst = np.empty(n, np.int64)
    pos = 0
    for sp in range(S):
        n_s = int(counts[sp])
        dst[order[pos : pos + n_s]] = off[sp] + np.arange(n_s)
        pos += n_s

    # species of each super tile (padding supers read species 0 weights)
    sup_species = np.zeros(T, np.int64)
    sup_species[:total_sup] = np.repeat(np.arange(S), sup_sp)

    # input planes [5, C, padded_n] fp16: s, vx, vy, vz, |v|^2
    f = np.asarray(node_feats, np.float32)
    planes = np.zeros((5, C, padded_n), np.float16)
    planes[0][:, dst] = f[:, :, 0].T
    planes[1][:, dst] = f[:, :, 1].T
    planes[2][:, dst] = f[:, :, 2].T
    planes[3][:, dst] = f[:, :, 3].T
    v2 = f[:, :, 1] ** 2 + f[:, :, 2] ** 2 + f[:, :, 3] ** 2
    planes[4][:, dst] = v2.T

    # per-super path-weight scalars [C, T*9] fp32
    w0 = np.asarray(w_paths0, np.float32)              # [S, 5, C]
    w1 = np.asarray(w_paths1, np.float32)              # [S, 4, C]
    wtab = np.concatenate([w0, w1], axis=1)            # [S, 9, C]
    wsel = wtab[sup_species].transpose(2, 0, 1).reshape(C, T * 9)

    wl0 = np.asarray(W_lin0, np.float16)
    wl1 = np.asarray(W_lin1, np.float16)

    nc = _get_nc(nsuper)
    in_maps = []
    for k in range(N_CORES):
        c0 = k * per_core
        in_maps.append(
            {
                "featsT": np.ascontiguousarray(planes[:, :, c0 : c0 + per_core]),
                "wsel": np.ascontiguousarray(
                    wsel[:, k * nsuper * 9 : (k + 1) * nsuper * 9]
                ),
                "wl0": wl0,
                "wl1": wl1,
            }
        )
    res = run_bass_kernel_spmd(nc, in_maps, core_ids=list(range(N_CORES)))
    # device layout [128, nsuper*NSUB, 4C]: node (ci,t,p) -> [p, ci*NSUB+t, :]
    outs = [
        res.results[k]["out"].transpose(1, 0, 2).reshape(per_core, 4 * C)
        for k in range(N_CORES)
    ]
    full = np.concatenate(outs, axis=0)[dst].astype(np.float32)  # [n, 512]

    y0 = full[:, :C]
    y1 = full[:, C:].reshape(n, 3, C).transpose(0, 2, 1).reshape(n, 3 * C)
    return np.ascontiguousarray(np.concatenate([y0, y1], axis=1))
